# revision 1
# baseline (speedup 1.0000x reference)
"""LLaMA attention block on 8 Trainium2 NeuronCores (Bass/Tile).

Problem: x[32,256,2048], wq/wk/wv/wo[2048,2048] fp32.
  q/k/v = x@W.T (per-head RoPE on q,k), causal softmax attention, y@wo.T.

Strategy:
- Data-parallel over batch: 8 cores x 4 batch elements.
- "Transposed" activation layout on-chip: activations kept as [C, T]
  (contraction dim on partitions) so every matmul is natural:
    proj:   out[o,t]   = sum_c wT[c,o] * xT[c,t]
    rope:   Rq = R @ q via a single 128x128 matmul per head tile
    scores: sT[k,q]    = sum_d krot[d,k] * qrot[d,q]   (scores transposed!)
    sum:    se[1,q]    = sum_k ones[k,1] * expm[k,q]   (ones-matmul)
    bcast:  rb[p,q]    = ones[1,p].T @ rinv[1,q]       (K=1 matmul)
    AV:     y[d,q]     = sum_k v[k,d] * expm[k,q]
    out:    out[o,t]   = sum_c woT[c,o] * yT[c,t]
  -> no on-chip transposes anywhere; softmax reduction via matmul.
- float32r (TF32-like, 11-bit mantissa, 1 cycle/row at N>=256) for all
  matmuls; fp32 accumulate in PSUM; softmax arithmetic in fp32.
- No max-subtraction in softmax: scores ~ N(0,1), exp is safe in fp32
  and matches the reference value-wise to fp32 rounding.
- Causal mask as a 0/1 multiply on exp(scores) before sum/AV.
- q/k/v spilled to DRAM scratch between projection and attention phases
  (weights are streamed column-block-wise once; SBUF can't hold all).
"""
import sys
sys.path.insert(0, '/opt/trn_rl_repo')
import math
import numpy as np

import concourse.bass as bass
import concourse.bacc as bacc
import concourse.mybir as mybir
import concourse.tile as tile
from concourse.bass_utils import run_bass_kernel_spmd

B, T, C = 32, 256, 2048
H, D = 16, 128
NCORES = 8
BPC = B // NCORES           # 4 batches per core
PAIRS = BPC // 2            # 2 batch-pairs (N=512 matmuls)
KT = C // 128               # 16 contraction tiles
OT = C // 128               # 16 output tiles
SCALE = 1.0 / math.sqrt(D)

F32 = mybir.dt.float32
F32R = mybir.dt.float32r
AF = mybir.ActivationFunctionType

_CACHE = {}


def _build():
    nc = bacc.Bacc("TRN2", target_bir_lowering=False, debug=False, num_devices=1)
    dt_in = {
        "xT": ([BPC, C, T], F32R),
        "wqT": ([C, C], F32R),
        "wkT": ([C, C], F32R),
        "wvT": ([C, C], F32R),
        "woT": ([C, C], F32R),
        "rmatT": ([128, 128], F32R),
        "cos2": ([128, 512], F32),
        "sin2": ([128, 512], F32),
        "mask2": ([128, 512], F32),
        "onescol": ([128, 1], F32R),
        "ones1x": ([1, 128], F32R),
    }
    aps = {n: nc.dram_tensor(n, s, d, kind="ExternalInput").ap()
           for n, (s, d) in dt_in.items()}
    out_d = nc.dram_tensor("out", [C, BPC * T], F32, kind="ExternalOutput").ap()

    with tile.TileContext(nc) as tc:
        from contextlib import ExitStack
        with ExitStack() as top:
            dpool = top.enter_context(tc.tile_pool(name="dram", bufs=1, space="DRAM"))
            qrot_d = dpool.tile([C, BPC * T], F32R, tag="qrot_d", name="qrot_d")
            krot_d = dpool.tile([C, BPC * T], F32R, tag="krot_d", name="krot_d")
            v_d = dpool.tile([BPC * T, C], F32R, tag="v_d", name="v_d")

            cpool = top.enter_context(tc.tile_pool(name="const", bufs=1))
            ct = {}
            for n in ("rmatT", "cos2", "sin2", "mask2", "onescol", "ones1x"):
                shape = dt_in[n][0]
                ct[n] = cpool.tile(list(shape), dt_in[n][1], tag=n, name=n)
                nc.sync.dma_start(ct[n][:], aps[n][:])

            # ---- load xT: 16 k-tiles of [128, BPC*T] ----
            with ExitStack() as proj_scope:
                xpool = proj_scope.enter_context(tc.tile_pool(name="xt", bufs=1))
                xtv = aps["xT"].rearrange("b (kt p) t -> kt p b t", p=128)
                xtiles = []
                for k in range(KT):
                    xt = xpool.tile([128, BPC * T], F32R, tag=f"xt{k}", name=f"xt{k}")
                    nc.sync.dma_start(xt[:].rearrange("p (b t) -> p b t", b=BPC), xtv[k])
                    xtiles.append(xt)

                # ---- Q and K projections + RoPE ----
                def proj_rope(wname, dst_d):
                    with ExitStack() as ph:
                        wp = ph.enter_context(tc.tile_pool(name=f"w{wname}", bufs=2))
                        sp = ph.enter_context(tc.tile_pool(name=f"s{wname}", bufs=3))
                        pp = ph.enter_context(tc.tile_pool(name=f"p{wname}", bufs=2, space="PSUM"))
                        pp2 = ph.enter_context(tc.tile_pool(name=f"p2{wname}", bufs=2, space="PSUM"))
                        wv_ = aps[wname]
                        for o in range(OT):
                            wblk = wp.tile([128, C], F32R, tag="wblk")
                            nc.sync.dma_start(
                                wblk[:].rearrange("p (kt m) -> p kt m", kt=KT),
                                wv_[:, o * 128:(o + 1) * 128]
                                .rearrange("(kt p) m -> p kt m", p=128))
                            for pr in range(PAIRS):
                                ps = pp.tile([128, 512], F32, tag="ps")
                                for k in range(KT):
                                    nc.tensor.matmul(
                                        ps[:], wblk[:, k * 128:(k + 1) * 128],
                                        xtiles[k][:, pr * 512:(pr + 1) * 512],
                                        start=(k == 0), stop=(k == KT - 1))
                                qs = sp.tile([128, 512], F32R, tag="qs")
                                nc.vector.tensor_copy(qs[:], ps[:])
                                rq = pp2.tile([128, 512], F32, tag="rq")
                                nc.tensor.matmul(rq[:], ct["rmatT"][:], qs[:],
                                                 start=True, stop=True)
                                t1 = sp.tile([128, 512], F32, tag="t1")
                                nc.vector.tensor_mul(t1[:], qs[:].bitcast(F32), ct["cos2"][:])
                                t2 = sp.tile([128, 512], F32, tag="t2")
                                nc.vector.tensor_mul(t2[:], rq[:], ct["sin2"][:])
                                qr = sp.tile([128, 512], F32R, tag="qr")
                                nc.vector.tensor_add(qr[:], t1[:], t2[:])
                                nc.sync.dma_start(
                                    dst_d[o * 128:(o + 1) * 128,
                                          pr * 512:(pr + 1) * 512], qr[:])

                proj_rope("wqT", qrot_d)
                proj_rope("wkT", krot_d)

                # ---- V projection: v in [T, C] layout ----
                with ExitStack() as ph:
                    wp = ph.enter_context(tc.tile_pool(name="wv", bufs=1))
                    sp = ph.enter_context(tc.tile_pool(name="sv", bufs=1))
                    pp = ph.enter_context(tc.tile_pool(name="pv", bufs=4, space="PSUM"))
                    vsb = [sp.tile([128, C], F32R, tag=f"vsb{i}", name=f"vsb{i}") for i in range(BPC * 2)]
                    for oc in range(4):
                        woc = wp.tile([128, KT * 512], F32R, tag="woc")
                        nc.sync.dma_start(
                            woc[:].rearrange("p (kt n) -> p kt n", kt=KT),
                            aps["wvT"][:, oc * 512:(oc + 1) * 512]
                            .rearrange("(kt p) n -> p kt n", p=128))
                        for b in range(BPC):
                            for tt in range(2):
                                ps = pp.tile([128, 512], F32, tag="psv")
                                for k in range(KT):
                                    nc.tensor.matmul(
                                        ps[:],
                                        xtiles[k][:, b * 256 + tt * 128:
                                                  b * 256 + (tt + 1) * 128],
                                        woc[:, k * 512:(k + 1) * 512],
                                        start=(k == 0), stop=(k == KT - 1))
                                nc.vector.tensor_copy(
                                    vsb[b * 2 + tt][:, oc * 512:(oc + 1) * 512], ps[:])
                    for i in range(BPC * 2):
                        nc.sync.dma_start(v_d[i * 128:(i + 1) * 128, :], vsb[i][:])

            # ---- attention per batch-pair ----
            ysb = []
            ypool = top.enter_context(tc.tile_pool(name="y", bufs=1))
            for pr in range(PAIRS):
                ysb.append([ypool.tile([128, 512], F32R, tag=f"y{pr}_{h}", name=f"y{pr}_{h}")
                            for h in range(H)])
            for pr in range(PAIRS):
                with ExitStack() as ph:
                    lp = ph.enter_context(tc.tile_pool(name=f"ld{pr}", bufs=1))
                    ap_ = ph.enter_context(tc.tile_pool(name=f"at{pr}", bufs=3))
                    scp = ph.enter_context(tc.tile_pool(name=f"sc{pr}", bufs=2, space="PSUM"))
                    sep = ph.enter_context(tc.tile_pool(name=f"se{pr}", bufs=2, space="PSUM"))
                    rbp = ph.enter_context(tc.tile_pool(name=f"rb{pr}", bufs=2, space="PSUM"))
                    yp = ph.enter_context(tc.tile_pool(name=f"yp{pr}", bufs=2, space="PSUM"))
                    qt, kt_ = [], []
                    for h in range(H):
                        q_ = lp.tile([128, 512], F32R, tag=f"q{h}")
                        nc.sync.dma_start(q_[:], qrot_d[h * 128:(h + 1) * 128,
                                                        pr * 512:(pr + 1) * 512])
                        qt.append(q_)
                        k_ = lp.tile([128, 512], F32R, tag=f"k{h}")
                        nc.sync.dma_start(k_[:], krot_d[h * 128:(h + 1) * 128,
                                                        pr * 512:(pr + 1) * 512])
                        kt_.append(k_)
                    vt = []
                    for i in range(4):  # 2 batches x 2 t-tiles
                        row = (pr * 2 + i // 2) * 2 + (i % 2)
                        v_ = lp.tile([128, C], F32R, tag=f"v{i}")
                        nc.sync.dma_start(v_[:], v_d[row * 128:(row + 1) * 128, :])
                        vt.append(v_)
                    for h in range(H):
                        for b in range(2):
                            sc = scp.tile([128, 512], F32, tag="sc")
                            for ktile in range(2):
                                nc.tensor.matmul(
                                    sc[:, ktile * 256:(ktile + 1) * 256],
                                    kt_[h][:, b * 256 + ktile * 128:
                                           b * 256 + (ktile + 1) * 128],
                                    qt[h][:, b * 256:(b + 1) * 256],
                                    start=True, stop=True)
                            ex = ap_.tile([128, 512], F32, tag="ex")
                            nc.scalar.activation(ex[:], sc[:], AF.Exp, scale=SCALE)
                            em = ap_.tile([128, 512], F32R, tag="em")
                            nc.vector.tensor_mul(em[:], ex[:], ct["mask2"][:])
                            se = sep.tile([1, 256], F32, tag="se")
                            nc.tensor.matmul(se[:], ct["onescol"][:], em[:, 0:256],
                                             start=True, stop=False)
                            nc.tensor.matmul(se[:], ct["onescol"][:], em[:, 256:512],
                                             start=False, stop=True)
                            rr = ap_.tile([1, 256], F32R, tag="rr")
                            with nc.allow_low_precision(reason="f32r rhs for bcast matmul"):
                                nc.vector.reciprocal(rr[:], se[:])
                            rb = rbp.tile([128, 256], F32, tag="rb")
                            nc.tensor.matmul(rb[:], ct["ones1x"][:], rr[:],
                                             start=True, stop=True)
                            rbs = ap_.tile([128, 256], F32, tag="rbs")
                            nc.scalar.activation(rbs[:], rb[:], AF.Identity)
                            yps = yp.tile([128, 256], F32, tag="yps")
                            vb = pr * 2 + b
                            nc.tensor.matmul(yps[:], vt[b * 2][:, h * 128:(h + 1) * 128],
                                             em[:, 0:256], start=True, stop=False)
                            nc.tensor.matmul(yps[:], vt[b * 2 + 1][:, h * 128:(h + 1) * 128],
                                             em[:, 256:512], start=False, stop=True)
                            nc.vector.tensor_mul(ysb[pr][h][:, b * 256:(b + 1) * 256],
                                                 yps[:], rbs[:])

            # ---- output projection ----
            with ExitStack() as ph:
                wp = ph.enter_context(tc.tile_pool(name="wo", bufs=2))
                sp = ph.enter_context(tc.tile_pool(name="so", bufs=3))
                pp = ph.enter_context(tc.tile_pool(name="po", bufs=4, space="PSUM"))
                for o in range(OT):
                    wblk = wp.tile([128, C], F32R, tag="wblk")
                    nc.sync.dma_start(
                        wblk[:].rearrange("p (kt m) -> p kt m", kt=KT),
                        aps["woT"][:, o * 128:(o + 1) * 128]
                        .rearrange("(kt p) m -> p kt m", p=128))
                    for pr in range(PAIRS):
                        ps = pp.tile([128, 512], F32, tag="pso")
                        for k in range(KT):
                            nc.tensor.matmul(ps[:], wblk[:, k * 128:(k + 1) * 128],
                                             ysb[pr][k][:],
                                             start=(k == 0), stop=(k == KT - 1))
                        os_ = sp.tile([128, 512], F32, tag="os")
                        nc.vector.tensor_copy(os_[:], ps[:])
                        nc.sync.dma_start(
                            out_d[o * 128:(o + 1) * 128,
                                  pr * 512:(pr + 1) * 512], os_[:])
    nc.compile()
    return nc


def _host_consts():
    inv_freq = 1.0 / (10000.0 ** (np.arange(0, D, 2, dtype=np.float32) / D))
    t = np.arange(T, dtype=np.float32)
    freqs = np.outer(t, inv_freq)                      # [T, 64]
    emb = np.concatenate([freqs, freqs], axis=-1)      # [T, 128]
    cosT = np.cos(emb).T.astype(np.float32)            # [128, 256]
    sinT = np.sin(emb).T.astype(np.float32)
    cos2 = np.concatenate([cosT, cosT], axis=1)        # [128, 512]
    sin2 = np.concatenate([sinT, sinT], axis=1)
    rmat = np.zeros((128, 128), dtype=np.float32)      # R @ q = rotate_half(q)
    for d in range(64):
        rmat[d, d + 64] = -1.0
        rmat[d + 64, d] = 1.0
    rmatT = rmat.T.copy()
    mask2 = np.zeros((128, 512), dtype=np.float32)
    k_idx = np.arange(128)[:, None]
    q_idx = np.arange(256)[None, :]
    mask2[:, 0:256] = (k_idx <= q_idx).astype(np.float32)
    mask2[:, 256:512] = ((k_idx + 128) <= q_idx).astype(np.float32)
    return {
        "cos2": cos2, "sin2": sin2, "rmatT": rmatT, "mask2": mask2,
        "onescol": np.ones((128, 1), np.float32),
        "ones1x": np.ones((1, 128), np.float32),
    }


def kernel(x, wq, wk, wv, wo):
    x = np.asarray(x, dtype=np.float32)
    if "nc" not in _CACHE:
        _CACHE["nc"] = _build()
    nc = _CACHE["nc"]
    consts = _host_consts()
    shared = {
        "wqT": np.ascontiguousarray(wq.T.astype(np.float32)),
        "wkT": np.ascontiguousarray(wk.T.astype(np.float32)),
        "wvT": np.ascontiguousarray(wv.T.astype(np.float32)),
        "woT": np.ascontiguousarray(wo.T.astype(np.float32)),
        **consts,
    }
    in_maps = []
    for c in range(NCORES):
        xs = x[c * BPC:(c + 1) * BPC]                  # [4, 256, 2048]
        xT = np.ascontiguousarray(xs.transpose(0, 2, 1))  # [4, 2048, 256]
        in_maps.append({"xT": xT, **shared})
    res = run_bass_kernel_spmd(nc, in_maps, core_ids=list(range(NCORES)))
    outs = []
    for c in range(NCORES):
        o = res.results[c]["out"]                      # [2048, 1024]
        o = o.reshape(C, PAIRS, 2, T)                  # [c, pair, b, t]
        o = o.transpose(1, 2, 3, 0).reshape(BPC, T, C)
        outs.append(o)
    return np.concatenate(outs, axis=0).astype(np.float32)



# revision 6
# speedup vs baseline: 9.5907x; 9.5907x over previous
"""LLaMA attention block on 8 Trainium2 NeuronCores (Bass/Tile).

Problem: x[32,256,2048], wq/wk/wv/wo[2048,2048] fp32.
  q/k/v = x@W.T (per-head RoPE on q,k), causal softmax attention, y@wo.T.

Strategy (v3):
- Data-parallel over batch: 8 cores x 4 batch elements (1024 tokens/core).
- bf16 operands everywhere on the matmul path (weights, x, q/k/v, exp-scores,
  y), fp32 PSUM accumulation. Measured end-to-end error vs fp64 ~6e-3,
  well under the 2e-2 gate. PE streams 1 col/cycle for bf16 == f32r, so this
  costs no PE time but halves DMA and SBUF traffic.
- No DRAM spill: q/k (rotated) live in small rotating SBUF pools; v and y
  (bf16, 4MB each) persist in SBUF. Per-head loop interleaves
  {q-proj, k-proj, attention} so attention overlaps the next head's
  projection matmuls and the PE never drains.
- All pools are allocated up-front at disjoint SBUF addresses: scoped/reused
  address ranges were observed (TimelineSim) to serialize weight prefetch
  behind the previous phase (a 4-8us PE gap per phase boundary).
- x is loaded as 16 separate tiles so the first V matmuls start after ~2us.
- Weights are pre-tiled on the host into the exact SBUF layout so every
  weight DMA is fully contiguous.
- "Transposed" activation layout on-chip ([contraction, tokens]); softmax
  reduction via ones-matmul; normalization via K=1 broadcast matmul.
- RoPE: rotate-half as one 128x128 matmul per tile; elementwise
  (q*cos + rq*sin) in bf16 on DVE (4x mode); PSUM drains on ScalarE.
- _build(loop_n=N) wraps the whole body in a tc.For_i hardware loop: one
  NEFF execution runs the kernel N times back-to-back. Used by test.py to
  measure pure on-device time as a slope between two loop counts,
  cancelling host/axon dispatch latency.
"""
import sys
sys.path.insert(0, '/opt/trn_rl_repo')
import math
import numpy as np
import ml_dtypes

import concourse.bass as bass
import concourse.bacc as bacc
import concourse.mybir as mybir
import concourse.tile as tile
from concourse.bass_utils import run_bass_kernel_spmd

B, T, C = 32, 256, 2048
H, D = 16, 128
NCORES = 8
BPC = B // NCORES           # 4 batches per core
PAIRS = BPC // 2            # 2 batch-pairs (N=512 matmuls)
KT = C // 128               # 16 contraction tiles
SCALE = 1.0 / math.sqrt(D)

F32 = mybir.dt.float32
F32R = mybir.dt.float32r
BF16 = mybir.dt.bfloat16
AF = mybir.ActivationFunctionType
NPBF = ml_dtypes.bfloat16

_CACHE = {}


def _build(loop_n: int = 1):
    nc = bacc.Bacc("TRN2", target_bir_lowering=False, debug=False, num_devices=1)
    dt_in = {
        "xt": ([KT, 128, BPC * T], BF16),      # [kt, p, (b t)]
        "wq": ([16, 128, KT * 128], BF16),     # [o_blk, p, (kt n)]
        "wk": ([16, 128, KT * 128], BF16),
        "wv": ([4, 128, KT * 512], BF16),      # [og, p, (kt n)]
        "wo": ([16, 128, KT * 128], BF16),
        "rmatT": ([128, 128], BF16),
        "cos2": ([128, 512], BF16),
        "sin2": ([128, 512], BF16),
        "mask2": ([128, 512], BF16),
        "onescol": ([128, 1], BF16),
        "ones1x": ([1, 128], F32R),
    }
    aps = {n: nc.dram_tensor(n, s, d, kind="ExternalInput").ap()
           for n, (s, d) in dt_in.items()}
    out_d = nc.dram_tensor("out", [C, BPC * T], F32, kind="ExternalOutput").ap()

    with tile.TileContext(nc) as tc:
        from contextlib import ExitStack
        with ExitStack() as top:
            # ---- constants (loaded once, outside any timing loop) ----
            cpool = top.enter_context(tc.tile_pool(name="const", bufs=1))
            ct = {}
            for n in ("rmatT", "cos2", "sin2", "mask2", "onescol", "ones1x"):
                shape = dt_in[n][0]
                ct[n] = cpool.tile(list(shape), dt_in[n][1], tag=n, name=n)
                nc.sync.dma_start(ct[n][:], aps[n][:])

            # ---- all pools up-front: disjoint SBUF regions ----
            xpool = top.enter_context(tc.tile_pool(name="xt", bufs=1))
            xtiles = [xpool.tile([128, BPC * T], BF16, tag=f"xt{kt}",
                                 name=f"xt{kt}") for kt in range(KT)]
            vpool = top.enter_context(tc.tile_pool(name="v", bufs=1))
            vsb = [vpool.tile([128, C], BF16, tag=f"v{i}", name=f"v{i}")
                   for i in range(BPC * 2)]       # [tok(128), all head dims]
            ypool = top.enter_context(tc.tile_pool(name="y", bufs=1))
            ysb = [ypool.tile([128, BPC * T], BF16, tag=f"y{h}", name=f"y{h}")
                   for h in range(H)]             # [head dims(128), tok]
            qkpool = top.enter_context(tc.tile_pool(name="qk", bufs=4))
            st = top.enter_context(tc.tile_pool(name="st", bufs=2))
            st3 = top.enter_context(tc.tile_pool(name="st3", bufs=3))
            vw = top.enter_context(tc.tile_pool(name="vw", bufs=4))
            qkw = top.enter_context(tc.tile_pool(name="qkw", bufs=3))
            ow = top.enter_context(tc.tile_pool(name="ow", bufs=2))

            # ---- PSUM pools: exactly 8 banks ----
            pp = top.enter_context(tc.tile_pool(name="pp", bufs=2, space="PSUM"))
            rp = top.enter_context(tc.tile_pool(name="rp", bufs=2, space="PSUM"))
            scp = top.enter_context(tc.tile_pool(name="scp", bufs=2, space="PSUM"))
            yp = top.enter_context(tc.tile_pool(name="yp", bufs=1, space="PSUM"))
            sep = top.enter_context(tc.tile_pool(name="sep", bufs=1, space="PSUM"))

            def emit_body():
                # First V matmul needs wv[0] first-half + xt[0]; queue those
                # bytes ahead of the rest of x so the PE starts ~3.5us in.
                wv_first = vw.tile([128, 8 * 512], BF16, tag="wvb")
                nc.sync.dma_start(wv_first[:], aps["wv"][0, :, 0:4096])
                for kt in range(KT):
                    nc.sync.dma_start(xtiles[kt][:], aps["xt"][kt])

                # ---- V projection: v in [tok, C] layout, 4-head groups,
                #      each og's weight split in two tiles for finer DMA deps
                for og in range(4):
                    wbs = []
                    for half in range(2):
                        if og == 0 and half == 0:
                            wbs.append(wv_first)
                            continue
                        wb = vw.tile([128, 8 * 512], BF16, tag="wvb")
                        nc.sync.dma_start(
                            wb[:], aps["wv"][og, :, half * 4096:(half + 1) * 4096])
                        wbs.append(wb)
                    for tt in range(BPC * 2):
                        ps = pp.tile([128, 512], F32, tag="pp")
                        for kt in range(KT):
                            nc.tensor.matmul(
                                ps[:],
                                xtiles[kt][:, tt * 128:(tt + 1) * 128],
                                wbs[kt // 8][:, (kt % 8) * 512:(kt % 8 + 1) * 512],
                                start=(kt == 0), stop=(kt == KT - 1))
                        nc.scalar.activation(
                            vsb[tt][:, og * 512:(og + 1) * 512], ps[:], AF.Identity)

                # ---- per-head: q-proj+rope, k-proj+rope, attention ----
                def proj_rope(wname, h, tag):
                    wb = qkw.tile([128, KT * 128], BF16, tag="wqk")
                    nc.sync.dma_start(wb[:], aps[wname][h])
                    dst = qkpool.tile([128, BPC * T], BF16, tag=tag)
                    for pr in range(PAIRS):
                        ps = pp.tile([128, 512], F32, tag="pp")
                        for kt in range(KT):
                            nc.tensor.matmul(
                                ps[:], wb[:, kt * 128:(kt + 1) * 128],
                                xtiles[kt][:, pr * 512:(pr + 1) * 512],
                                start=(kt == 0), stop=(kt == KT - 1))
                        qs = st3.tile([128, 512], BF16, tag="qs")
                        nc.scalar.activation(qs[:], ps[:], AF.Identity)
                        rq = rp.tile([128, 512], F32, tag="rq")
                        nc.tensor.matmul(rq[:], ct["rmatT"][:], qs[:],
                                         start=True, stop=True)
                        rqs = st3.tile([128, 512], BF16, tag="rqs")
                        nc.scalar.activation(rqs[:], rq[:], AF.Identity)
                        t1 = st.tile([128, 512], BF16, tag="t1")
                        nc.vector.tensor_mul(t1[:], qs[:], ct["cos2"][:])
                        t2 = st.tile([128, 512], BF16, tag="t2")
                        nc.vector.tensor_mul(t2[:], rqs[:], ct["sin2"][:])
                        nc.vector.tensor_add(
                            dst[:, pr * 512:(pr + 1) * 512], t1[:], t2[:])
                    return dst

                for h in range(H):
                    qh = proj_rope("wq", h, "q")
                    kh = proj_rope("wk", h, "k")
                    for b in range(BPC):
                        # Causal skip: ktile1 (keys 128..255) is fully masked
                        # for queries 0..127, so score/exp/AV only cover
                        # [k0 x q-all | k1 x q-high] = 384 columns, not 512.
                        # em free layout: [0:256] = k-tile0 vs all 256 q;
                        # [256:384] = k-tile1 vs q in [128,256).
                        sc = scp.tile([128, 384], F32, tag="sc")
                        nc.tensor.matmul(
                            sc[:, 0:256],
                            kh[:, b * 256:b * 256 + 128],
                            qh[:, b * 256:(b + 1) * 256],
                            start=True, stop=True)
                        nc.tensor.matmul(
                            sc[:, 256:384],
                            kh[:, b * 256 + 128:b * 256 + 256],
                            qh[:, b * 256 + 128:b * 256 + 256],
                            start=True, stop=True)
                        ex = st3.tile([128, 384], BF16, tag="ex")
                        nc.scalar.activation(ex[:], sc[:], AF.Exp, scale=SCALE)
                        em = st3.tile([128, 384], BF16, tag="em")
                        tri = ct["mask2"][:, 0:128]       # k<=q triangle
                        nc.vector.tensor_mul(em[:, 0:128], ex[:, 0:128], tri)
                        nc.vector.tensor_copy(em[:, 128:256], ex[:, 128:256])
                        nc.vector.tensor_mul(em[:, 256:384], ex[:, 256:384], tri)
                        se = sep.tile([1, 256], F32, tag="serb")
                        nc.tensor.matmul(se[:], ct["onescol"][:], em[:, 0:256],
                                         start=True, stop=False)
                        nc.tensor.matmul(se[:, 128:256], ct["onescol"][:],
                                         em[:, 256:384],
                                         start=False, stop=True)
                        rr = st.tile([1, 256], F32R, tag="rr")
                        with nc.allow_low_precision(reason="f32r rhs for bcast mm"):
                            nc.vector.reciprocal(rr[:], se[:])
                        rb = sep.tile([128, 256], F32, tag="serb")
                        nc.tensor.matmul(rb[:], ct["ones1x"][:], rr[:],
                                         start=True, stop=True)
                        rbs = st.tile([128, 256], F32, tag="rbs")
                        nc.scalar.activation(rbs[:], rb[:], AF.Identity)
                        yps = yp.tile([128, 256], F32, tag="yps")
                        nc.tensor.matmul(yps[:], vsb[b * 2][:, h * 128:(h + 1) * 128],
                                         em[:, 0:256], start=True, stop=False)
                        nc.tensor.matmul(yps[:, 128:256],
                                         vsb[b * 2 + 1][:, h * 128:(h + 1) * 128],
                                         em[:, 256:384], start=False, stop=True)
                        nc.vector.tensor_mul(ysb[h][:, b * 256:(b + 1) * 256],
                                             yps[:], rbs[:])

                # ---- output projection ----
                for o in range(16):
                    wb = ow.tile([128, KT * 128], BF16, tag="wob")
                    nc.sync.dma_start(wb[:], aps["wo"][o])
                    for pr in range(PAIRS):
                        ps = pp.tile([128, 512], F32, tag="pp")
                        for c in range(KT):
                            nc.tensor.matmul(
                                ps[:], wb[:, c * 128:(c + 1) * 128],
                                ysb[c][:, pr * 512:(pr + 1) * 512],
                                start=(c == 0), stop=(c == KT - 1))
                        os_ = st3.tile([128, 512], F32, tag="os")
                        nc.scalar.activation(os_[:], ps[:], AF.Identity)
                        nc.sync.dma_start(
                            out_d[o * 128:(o + 1) * 128,
                                  pr * 512:(pr + 1) * 512], os_[:])

            if loop_n == 1:
                emit_body()
            else:
                with tc.For_i(0, loop_n, 1):
                    emit_body()
    nc.compile()
    return nc


def _host_consts():
    inv_freq = 1.0 / (10000.0 ** (np.arange(0, D, 2, dtype=np.float32) / D))
    t = np.arange(T, dtype=np.float32)
    freqs = np.outer(t, inv_freq)                      # [T, 64]
    emb = np.concatenate([freqs, freqs], axis=-1)      # [T, 128]
    cosT = np.cos(emb).T.astype(np.float32)            # [128, 256]
    sinT = np.sin(emb).T.astype(np.float32)
    cos2 = np.concatenate([cosT, cosT], axis=1)        # [128, 512]
    sin2 = np.concatenate([sinT, sinT], axis=1)
    rmat = np.zeros((128, 128), dtype=np.float32)      # R @ q = rotate_half(q)
    for d in range(64):
        rmat[d, d + 64] = -1.0
        rmat[d + 64, d] = 1.0
    rmatT = rmat.T.copy()
    mask2 = np.zeros((128, 512), dtype=np.float32)
    k_idx = np.arange(128)[:, None]
    q_idx = np.arange(256)[None, :]
    mask2[:, 0:256] = (k_idx <= q_idx).astype(np.float32)
    mask2[:, 256:512] = ((k_idx + 128) <= q_idx).astype(np.float32)
    return {
        "cos2": cos2.astype(NPBF), "sin2": sin2.astype(NPBF),
        "rmatT": rmatT.astype(NPBF), "mask2": mask2.astype(NPBF),
        "onescol": np.ones((128, 1), NPBF),
        "ones1x": np.ones((1, 128), np.float32),
    }


def _tile_w(w, blk):
    """w [2048,2048] fp32 -> [2048//blk, 128, 16*blk] bf16 with
    out[ob, p, kt*blk + n] = w[ob*blk + n, kt*128 + p]."""
    nblk = 2048 // blk
    t = w.reshape(nblk, blk, KT, 128).transpose(0, 3, 2, 1)
    return np.ascontiguousarray(t.reshape(nblk, 128, KT * blk)).astype(NPBF)


def _host_weights(wq, wk, wv, wo):
    return {
        "wq": _tile_w(np.asarray(wq, np.float32), 128),
        "wk": _tile_w(np.asarray(wk, np.float32), 128),
        "wv": _tile_w(np.asarray(wv, np.float32), 512),
        "wo": _tile_w(np.asarray(wo, np.float32), 128),
    }


def _host_x(x):
    """x [B,T,C] fp32 -> per-core [KT, 128, BPC*T] bf16 list."""
    xb = np.asarray(x, np.float32).astype(NPBF)
    outs = []
    for c in range(NCORES):
        xs = xb[c * BPC:(c + 1) * BPC]                      # [4,256,2048]
        xt = xs.transpose(2, 0, 1).reshape(KT, 128, BPC * T)
        outs.append(np.ascontiguousarray(xt))
    return outs


def kernel(x, wq, wk, wv, wo):
    if "nc" not in _CACHE:
        _CACHE["nc"] = _build()
    nc = _CACHE["nc"]
    shared = {**_host_weights(wq, wk, wv, wo), **_host_consts()}
    xts = _host_x(x)
    in_maps = [{"xt": xts[c], **shared} for c in range(NCORES)]
    res = run_bass_kernel_spmd(nc, in_maps, core_ids=list(range(NCORES)))
    outs = []
    for c in range(NCORES):
        o = res.results[c]["out"]                      # [2048, 1024]
        o = o.reshape(C, PAIRS, 2, T)                  # [c, pair, b, t]
        o = o.transpose(1, 2, 3, 0).reshape(BPC, T, C)
        outs.append(o)
    return np.concatenate(outs, axis=0).astype(np.float32)


# revision 10
# speedup vs baseline: 9.6209x; 1.0031x over previous
"""LLaMA attention block on 8 Trainium2 NeuronCores (Bass/Tile).

Problem: x[32,256,2048], wq/wk/wv/wo[2048,2048] fp32.
  q/k/v = x@W.T (per-head RoPE on q,k), causal softmax attention, y@wo.T.

Strategy (v3):
- Data-parallel over batch: 8 cores x 4 batch elements (1024 tokens/core).
- bf16 operands everywhere on the matmul path (weights, x, q/k/v, exp-scores,
  y), fp32 PSUM accumulation. Measured end-to-end error vs fp64 ~6e-3,
  well under the 2e-2 gate. PE streams 1 col/cycle for bf16 == f32r, so this
  costs no PE time but halves DMA and SBUF traffic.
- No DRAM spill: q/k (rotated) live in small rotating SBUF pools; v and y
  (bf16, 4MB each) persist in SBUF. Per-head loop interleaves
  {q-proj, k-proj, attention} so attention overlaps the next head's
  projection matmuls and the PE never drains.
- All pools are allocated up-front at disjoint SBUF addresses: scoped/reused
  address ranges were observed (TimelineSim) to serialize weight prefetch
  behind the previous phase (a 4-8us PE gap per phase boundary).
- x is loaded as 16 separate tiles so the first V matmuls start after ~2us.
- Weights are pre-tiled on the host into the exact SBUF layout so every
  weight DMA is fully contiguous.
- "Transposed" activation layout on-chip ([contraction, tokens]); softmax
  reduction via ones-matmul; normalization via K=1 broadcast matmul.
- RoPE: rotate-half as one 128x128 matmul per tile; elementwise
  (q*cos + rq*sin) in bf16 on DVE (4x mode); PSUM drains on ScalarE.
- _build(loop_n=N) wraps the whole body in a tc.For_i hardware loop: one
  NEFF execution runs the kernel N times back-to-back. Used by test.py to
  measure pure on-device time as a slope between two loop counts,
  cancelling host/axon dispatch latency.
"""
import sys
sys.path.insert(0, '/opt/trn_rl_repo')
import math
import numpy as np
import ml_dtypes

import concourse.bass as bass
import concourse.bacc as bacc
import concourse.mybir as mybir
import concourse.tile as tile
from concourse.bass_utils import run_bass_kernel_spmd

B, T, C = 32, 256, 2048
H, D = 16, 128
NCORES = 8
BPC = B // NCORES           # 4 batches per core
PAIRS = BPC // 2            # 2 batch-pairs (N=512 matmuls)
KT = C // 128               # 16 contraction tiles
SCALE = 1.0 / math.sqrt(D)

F32 = mybir.dt.float32
F32R = mybir.dt.float32r
BF16 = mybir.dt.bfloat16
AF = mybir.ActivationFunctionType
NPBF = ml_dtypes.bfloat16

_CACHE = {}


def _build(loop_n: int = 1):
    nc = bacc.Bacc("TRN2", target_bir_lowering=False, debug=False, num_devices=1)
    dt_in = {
        "xt": ([KT, 128, BPC * T], BF16),      # [kt, p, (b t)]
        "wq": ([16, 128, KT * 128], BF16),     # [o_blk, p, (kt n)]
        "wk": ([16, 128, KT * 128], BF16),
        "wv": ([4, 128, KT * 512], BF16),      # [og, p, (kt n)]
        "wo": ([16, 128, KT * 128], BF16),
        "rmatT": ([128, 128], BF16),
        "cos2": ([128, 512], BF16),
        "sin2": ([128, 512], BF16),
        "mask2": ([128, 512], BF16),
        "onescol": ([128, 1], BF16),
        "ones1x": ([1, 128], F32R),
    }
    aps = {n: nc.dram_tensor(n, s, d, kind="ExternalInput").ap()
           for n, (s, d) in dt_in.items()}
    out_d = nc.dram_tensor("out", [C, BPC * T], F32, kind="ExternalOutput").ap()

    with tile.TileContext(nc) as tc:
        from contextlib import ExitStack
        with ExitStack() as top:
            # ---- constants (loaded once, outside any timing loop) ----
            cpool = top.enter_context(tc.tile_pool(name="const", bufs=1))
            ct = {}
            for n in ("rmatT", "cos2", "sin2", "mask2", "onescol", "ones1x"):
                shape = dt_in[n][0]
                ct[n] = cpool.tile(list(shape), dt_in[n][1], tag=n, name=n)
                nc.sync.dma_start(ct[n][:], aps[n][:])

            # ---- all pools up-front: disjoint SBUF regions ----
            xpool = top.enter_context(tc.tile_pool(name="xt", bufs=1))
            xtiles = [xpool.tile([128, BPC * T], BF16, tag=f"xt{kt}",
                                 name=f"xt{kt}") for kt in range(KT)]
            vpool = top.enter_context(tc.tile_pool(name="v", bufs=1))
            vsb = [vpool.tile([128, C], BF16, tag=f"v{i}", name=f"v{i}")
                   for i in range(BPC * 2)]       # [tok(128), all head dims]
            ypool = top.enter_context(tc.tile_pool(name="y", bufs=1))
            ysb = [ypool.tile([128, BPC * T], BF16, tag=f"y{h}", name=f"y{h}")
                   for h in range(H)]             # [head dims(128), tok]
            qkpool = top.enter_context(tc.tile_pool(name="qk", bufs=4))
            st = top.enter_context(tc.tile_pool(name="st", bufs=2))
            st3 = top.enter_context(tc.tile_pool(name="st3", bufs=3))
            vw = top.enter_context(tc.tile_pool(name="vw", bufs=4))
            qkw = top.enter_context(tc.tile_pool(name="qkw", bufs=3))
            ow = top.enter_context(tc.tile_pool(name="ow", bufs=2))

            # ---- PSUM pools: exactly 8 banks ----
            pp = top.enter_context(tc.tile_pool(name="pp", bufs=3, space="PSUM"))
            rp = top.enter_context(tc.tile_pool(name="rp", bufs=2, space="PSUM"))
            scp = top.enter_context(tc.tile_pool(name="scp", bufs=1, space="PSUM"))
            yp = top.enter_context(tc.tile_pool(name="yp", bufs=1, space="PSUM"))
            sep = top.enter_context(tc.tile_pool(name="sep", bufs=1, space="PSUM"))

            def emit_body():
                # First V matmul needs wv[0] first-half + xt[0]; queue those
                # bytes ahead of the rest of x so the PE starts ~3.5us in.
                wv_first = vw.tile([128, 8 * 512], BF16, tag="wvb")
                nc.sync.dma_start(wv_first[:], aps["wv"][0, :, 0:4096])
                for kt in range(KT):
                    nc.sync.dma_start(xtiles[kt][:], aps["xt"][kt])

                # ---- V projection: v in [tok, C] layout, 4-head groups,
                #      each og's weight split in two tiles for finer DMA deps
                for og in range(4):
                    wbs = []
                    for half in range(2):
                        if og == 0 and half == 0:
                            wbs.append(wv_first)
                            continue
                        wb = vw.tile([128, 8 * 512], BF16, tag="wvb")
                        nc.sync.dma_start(
                            wb[:], aps["wv"][og, :, half * 4096:(half + 1) * 4096])
                        wbs.append(wb)
                    for tt in range(BPC * 2):
                        ps = pp.tile([128, 512], F32, tag="pp")
                        for kt in range(KT):
                            nc.tensor.matmul(
                                ps[:],
                                xtiles[kt][:, tt * 128:(tt + 1) * 128],
                                wbs[kt // 8][:, (kt % 8) * 512:(kt % 8 + 1) * 512],
                                start=(kt == 0), stop=(kt == KT - 1))
                        nc.scalar.activation(
                            vsb[tt][:, og * 512:(og + 1) * 512], ps[:], AF.Identity)

                # ---- per-head: q-proj+rope, k-proj+rope, attention ----
                def proj_rope(wname, h, tag):
                    wb = qkw.tile([128, KT * 128], BF16, tag="wqk")
                    nc.sync.dma_start(wb[:], aps[wname][h])
                    dst = qkpool.tile([128, BPC * T], BF16, tag=tag)
                    # kt-outer so each 128-col weight slice is loaded into the
                    # PE once and streamed against both batch-pairs.
                    pss = [pp.tile([128, 512], F32, tag="pp", name=f"ps{pr}")
                           for pr in range(PAIRS)]
                    for kt in range(KT):
                        for pr in range(PAIRS):
                            nc.tensor.matmul(
                                pss[pr][:], wb[:, kt * 128:(kt + 1) * 128],
                                xtiles[kt][:, pr * 512:(pr + 1) * 512],
                                start=(kt == 0), stop=(kt == KT - 1))
                    for pr in range(PAIRS):
                        ps = pss[pr]
                        qs = st3.tile([128, 512], BF16, tag="qs")
                        nc.scalar.activation(qs[:], ps[:], AF.Identity)
                        rq = rp.tile([128, 512], F32, tag="rq")
                        nc.tensor.matmul(rq[:], ct["rmatT"][:], qs[:],
                                         start=True, stop=True)
                        rqs = st3.tile([128, 512], BF16, tag="rqs")
                        nc.scalar.activation(rqs[:], rq[:], AF.Identity)
                        t1 = st.tile([128, 512], BF16, tag="t1")
                        nc.vector.tensor_mul(t1[:], qs[:], ct["cos2"][:])
                        t2 = st.tile([128, 512], BF16, tag="t2")
                        nc.vector.tensor_mul(t2[:], rqs[:], ct["sin2"][:])
                        nc.vector.tensor_add(
                            dst[:, pr * 512:(pr + 1) * 512], t1[:], t2[:])
                    return dst

                for h in range(H):
                    qh = proj_rope("wq", h, "q")
                    kh = proj_rope("wk", h, "k")
                    for b in range(BPC):
                        # Causal skip: ktile1 (keys 128..255) is fully masked
                        # for queries 0..127, so score/exp/AV only cover
                        # [k0 x q-all | k1 x q-high] = 384 columns, not 512.
                        # em free layout: [0:256] = k-tile0 vs all 256 q;
                        # [256:384] = k-tile1 vs q in [128,256).
                        sc = scp.tile([128, 384], F32, tag="sc")
                        nc.tensor.matmul(
                            sc[:, 0:256],
                            kh[:, b * 256:b * 256 + 128],
                            qh[:, b * 256:(b + 1) * 256],
                            start=True, stop=True)
                        nc.tensor.matmul(
                            sc[:, 256:384],
                            kh[:, b * 256 + 128:b * 256 + 256],
                            qh[:, b * 256 + 128:b * 256 + 256],
                            start=True, stop=True)
                        ex = st3.tile([128, 384], BF16, tag="ex")
                        nc.scalar.activation(ex[:], sc[:], AF.Exp, scale=SCALE)
                        em = st3.tile([128, 384], BF16, tag="em")
                        tri = ct["mask2"][:, 0:128]       # k<=q triangle
                        nc.vector.tensor_mul(em[:, 0:128], ex[:, 0:128], tri)
                        nc.vector.tensor_copy(em[:, 128:256], ex[:, 128:256])
                        nc.vector.tensor_mul(em[:, 256:384], ex[:, 256:384], tri)
                        se = sep.tile([1, 256], F32, tag="serb")
                        nc.tensor.matmul(se[:], ct["onescol"][:], em[:, 0:256],
                                         start=True, stop=False)
                        nc.tensor.matmul(se[:, 128:256], ct["onescol"][:],
                                         em[:, 256:384],
                                         start=False, stop=True)
                        rr = st.tile([1, 256], F32R, tag="rr")
                        with nc.allow_low_precision(reason="f32r rhs for bcast mm"):
                            nc.vector.reciprocal(rr[:], se[:])
                        rb = sep.tile([128, 256], F32, tag="serb")
                        nc.tensor.matmul(rb[:], ct["ones1x"][:], rr[:],
                                         start=True, stop=True)
                        rbs = st.tile([128, 256], F32, tag="rbs")
                        nc.scalar.activation(rbs[:], rb[:], AF.Identity)
                        yps = yp.tile([128, 256], F32, tag="yps")
                        nc.tensor.matmul(yps[:], vsb[b * 2][:, h * 128:(h + 1) * 128],
                                         em[:, 0:256], start=True, stop=False)
                        nc.tensor.matmul(yps[:, 128:256],
                                         vsb[b * 2 + 1][:, h * 128:(h + 1) * 128],
                                         em[:, 256:384], start=False, stop=True)
                        nc.vector.tensor_mul(ysb[h][:, b * 256:(b + 1) * 256],
                                             yps[:], rbs[:])

                # ---- output projection ----
                for o in range(16):
                    wb = ow.tile([128, KT * 128], BF16, tag="wob")
                    nc.sync.dma_start(wb[:], aps["wo"][o])
                    pss = [pp.tile([128, 512], F32, tag="pp", name=f"pso{pr}")
                           for pr in range(PAIRS)]
                    for c in range(KT):
                        for pr in range(PAIRS):
                            nc.tensor.matmul(
                                pss[pr][:], wb[:, c * 128:(c + 1) * 128],
                                ysb[c][:, pr * 512:(pr + 1) * 512],
                                start=(c == 0), stop=(c == KT - 1))
                    for pr in range(PAIRS):
                        os_ = st3.tile([128, 512], F32, tag="os")
                        nc.scalar.activation(os_[:], pss[pr][:], AF.Identity)
                        nc.sync.dma_start(
                            out_d[o * 128:(o + 1) * 128,
                                  pr * 512:(pr + 1) * 512], os_[:])

            if loop_n == 1:
                emit_body()
            else:
                with tc.For_i(0, loop_n, 1):
                    emit_body()
    nc.compile()
    return nc


def _host_consts():
    inv_freq = 1.0 / (10000.0 ** (np.arange(0, D, 2, dtype=np.float32) / D))
    t = np.arange(T, dtype=np.float32)
    freqs = np.outer(t, inv_freq)                      # [T, 64]
    emb = np.concatenate([freqs, freqs], axis=-1)      # [T, 128]
    cosT = np.cos(emb).T.astype(np.float32)            # [128, 256]
    sinT = np.sin(emb).T.astype(np.float32)
    cos2 = np.concatenate([cosT, cosT], axis=1)        # [128, 512]
    sin2 = np.concatenate([sinT, sinT], axis=1)
    rmat = np.zeros((128, 128), dtype=np.float32)      # R @ q = rotate_half(q)
    for d in range(64):
        rmat[d, d + 64] = -1.0
        rmat[d + 64, d] = 1.0
    rmatT = rmat.T.copy()
    mask2 = np.zeros((128, 512), dtype=np.float32)
    k_idx = np.arange(128)[:, None]
    q_idx = np.arange(256)[None, :]
    mask2[:, 0:256] = (k_idx <= q_idx).astype(np.float32)
    mask2[:, 256:512] = ((k_idx + 128) <= q_idx).astype(np.float32)
    return {
        "cos2": cos2.astype(NPBF), "sin2": sin2.astype(NPBF),
        "rmatT": rmatT.astype(NPBF), "mask2": mask2.astype(NPBF),
        "onescol": np.ones((128, 1), NPBF),
        "ones1x": np.ones((1, 128), np.float32),
    }


def _tile_w(w, blk):
    """w [2048,2048] fp32 -> [2048//blk, 128, 16*blk] bf16 with
    out[ob, p, kt*blk + n] = w[ob*blk + n, kt*128 + p]."""
    nblk = 2048 // blk
    t = w.reshape(nblk, blk, KT, 128).transpose(0, 3, 2, 1)
    return np.ascontiguousarray(t.reshape(nblk, 128, KT * blk)).astype(NPBF)


def _host_weights(wq, wk, wv, wo):
    return {
        "wq": _tile_w(np.asarray(wq, np.float32), 128),
        "wk": _tile_w(np.asarray(wk, np.float32), 128),
        "wv": _tile_w(np.asarray(wv, np.float32), 512),
        "wo": _tile_w(np.asarray(wo, np.float32), 128),
    }


def _host_x(x):
    """x [B,T,C] fp32 -> per-core [KT, 128, BPC*T] bf16 list."""
    xb = np.asarray(x, np.float32).astype(NPBF)
    outs = []
    for c in range(NCORES):
        xs = xb[c * BPC:(c + 1) * BPC]                      # [4,256,2048]
        xt = xs.transpose(2, 0, 1).reshape(KT, 128, BPC * T)
        outs.append(np.ascontiguousarray(xt))
    return outs


def kernel(x, wq, wk, wv, wo):
    if "nc" not in _CACHE:
        _CACHE["nc"] = _build()
    nc = _CACHE["nc"]
    shared = {**_host_weights(wq, wk, wv, wo), **_host_consts()}
    xts = _host_x(x)
    in_maps = [{"xt": xts[c], **shared} for c in range(NCORES)]
    res = run_bass_kernel_spmd(nc, in_maps, core_ids=list(range(NCORES)))
    outs = []
    for c in range(NCORES):
        o = res.results[c]["out"]                      # [2048, 1024]
        o = o.reshape(C, PAIRS, 2, T)                  # [c, pair, b, t]
        o = o.transpose(1, 2, 3, 0).reshape(BPC, T, C)
        outs.append(o)
    return np.concatenate(outs, axis=0).astype(np.float32)


# revision 13
# speedup vs baseline: 10.3888x; 1.0798x over previous
"""LLaMA attention block on 8 Trainium2 NeuronCores (Bass/Tile).

Problem: x[32,256,2048], wq/wk/wv/wo[2048,2048] fp32.
  q/k/v = x@W.T (per-head RoPE on q,k), causal softmax attention, y@wo.T.

Strategy (v3):
- Data-parallel over batch: 8 cores x 4 batch elements (1024 tokens/core).
- bf16 operands everywhere on the matmul path (weights, x, q/k/v, exp-scores,
  y), fp32 PSUM accumulation. Measured end-to-end error vs fp64 ~6e-3,
  well under the 2e-2 gate. PE streams 1 col/cycle for bf16 == f32r, so this
  costs no PE time but halves DMA and SBUF traffic.
- No DRAM spill: q/k (rotated) live in small rotating SBUF pools; v and y
  (bf16, 4MB each) persist in SBUF. Per-head loop interleaves
  {q-proj, k-proj, attention} so attention overlaps the next head's
  projection matmuls and the PE never drains.
- All pools are allocated up-front at disjoint SBUF addresses: scoped/reused
  address ranges were observed (TimelineSim) to serialize weight prefetch
  behind the previous phase (a 4-8us PE gap per phase boundary).
- x is loaded as 16 separate tiles so the first V matmuls start after ~2us.
- Weights are pre-tiled on the host into the exact SBUF layout so every
  weight DMA is fully contiguous.
- "Transposed" activation layout on-chip ([contraction, tokens]); softmax
  reduction via ones-matmul; normalization via K=1 broadcast matmul.
- RoPE: rotate-half as one 128x128 matmul per tile; elementwise
  (q*cos + rq*sin) in bf16 on DVE (4x mode); PSUM drains on ScalarE.
- _build(loop_n=N) wraps the whole body in a tc.For_i hardware loop: one
  NEFF execution runs the kernel N times back-to-back. Used by test.py to
  measure pure on-device time as a slope between two loop counts,
  cancelling host/axon dispatch latency.
"""
import sys
sys.path.insert(0, '/opt/trn_rl_repo')
import math
import numpy as np
import ml_dtypes

import concourse.bass as bass
import concourse.bacc as bacc
import concourse.mybir as mybir
import concourse.tile as tile
from concourse.bass_utils import run_bass_kernel_spmd

B, T, C = 32, 256, 2048
H, D = 16, 128
NCORES = 8
BPC = B // NCORES           # 4 batches per core
PAIRS = BPC // 2            # 2 batch-pairs (N=512 matmuls)
KT = C // 128               # 16 contraction tiles
SCALE = 1.0 / math.sqrt(D)

F32 = mybir.dt.float32
F32R = mybir.dt.float32r
BF16 = mybir.dt.bfloat16
AF = mybir.ActivationFunctionType
NPBF = ml_dtypes.bfloat16

_CACHE = {}


def _build(loop_n: int = 1):
    nc = bacc.Bacc("TRN2", target_bir_lowering=False, debug=False, num_devices=1)
    dt_in = {
        "xt": ([KT, 128, BPC * T], BF16),      # [kt, p, (b t)]
        "wq": ([16, 128, KT * 128], BF16),     # [o_blk, p, (kt n)]
        "wk": ([16, 128, KT * 128], BF16),
        "wv": ([4, 128, KT * 512], BF16),      # [og, p, (kt n)]
        "wo": ([16, 128, KT * 128], BF16),
        "rmatT": ([128, 128], BF16),
        "cos2": ([128, 512], BF16),
        "sin2": ([128, 512], BF16),
        "mask2": ([128, 512], BF16),
        "onescol": ([128, 1], BF16),
        "ones1x": ([1, 128], F32R),
    }
    aps = {n: nc.dram_tensor(n, s, d, kind="ExternalInput").ap()
           for n, (s, d) in dt_in.items()}
    out_d = nc.dram_tensor("out", [C, BPC * T], F32, kind="ExternalOutput").ap()

    with tile.TileContext(nc) as tc:
        from contextlib import ExitStack
        with ExitStack() as top:
            # ---- constants (loaded once, outside any timing loop) ----
            cpool = top.enter_context(tc.tile_pool(name="const", bufs=1))
            ct = {}
            for n in ("rmatT", "cos2", "sin2", "mask2", "onescol", "ones1x"):
                shape = dt_in[n][0]
                ct[n] = cpool.tile(list(shape), dt_in[n][1], tag=n, name=n)
                nc.sync.dma_start(ct[n][:], aps[n][:])

            # ---- all pools up-front: disjoint SBUF regions ----
            xpool = top.enter_context(tc.tile_pool(name="xt", bufs=1))
            xtiles = [xpool.tile([128, BPC * T], BF16, tag=f"xt{kt}",
                                 name=f"xt{kt}") for kt in range(KT)]
            vpool = top.enter_context(tc.tile_pool(name="v", bufs=1))
            vsb = [vpool.tile([128, C], BF16, tag=f"v{i}", name=f"v{i}")
                   for i in range(BPC * 2)]       # [tok(128), all head dims]
            ypool = top.enter_context(tc.tile_pool(name="y", bufs=1))
            ysb = [ypool.tile([128, BPC * T], BF16, tag=f"y{h}", name=f"y{h}")
                   for h in range(H)]             # [head dims(128), tok]
            qkpool = top.enter_context(tc.tile_pool(name="qk", bufs=4))
            st = top.enter_context(tc.tile_pool(name="st", bufs=2))
            st3 = top.enter_context(tc.tile_pool(name="st3", bufs=3))
            vw = top.enter_context(tc.tile_pool(name="vw", bufs=4))
            qkw = top.enter_context(tc.tile_pool(name="qkw", bufs=3))
            ow = top.enter_context(tc.tile_pool(name="ow", bufs=2))

            # ---- PSUM pools: exactly 8 banks ----
            pp = top.enter_context(tc.tile_pool(name="pp", bufs=3, space="PSUM"))
            rp = top.enter_context(tc.tile_pool(name="rp", bufs=2, space="PSUM"))
            scp = top.enter_context(tc.tile_pool(name="scp", bufs=1, space="PSUM"))
            yp = top.enter_context(tc.tile_pool(name="yp", bufs=1, space="PSUM"))
            sep = top.enter_context(tc.tile_pool(name="sep", bufs=1, space="PSUM"))

            def emit_body():
                # First V matmul needs wv[0] first-half + xt[0]; queue those
                # bytes ahead of the rest of x so the PE starts ~3.5us in.
                wv_first = vw.tile([128, 8 * 512], BF16, tag="wvb")
                nc.sync.dma_start(wv_first[:], aps["wv"][0, :, 0:4096])
                for kt in range(KT):
                    nc.sync.dma_start(xtiles[kt][:], aps["xt"][kt])

                # ---- V projection: v in [tok, C] layout, 4-head groups,
                #      each og's weight split in two tiles for finer DMA deps
                for og in range(4):
                    wbs = []
                    for half in range(2):
                        if og == 0 and half == 0:
                            wbs.append(wv_first)
                            continue
                        wb = vw.tile([128, 8 * 512], BF16, tag="wvb")
                        nc.sync.dma_start(
                            wb[:], aps["wv"][og, :, half * 4096:(half + 1) * 4096])
                        wbs.append(wb)
                    for tt in range(BPC * 2):
                        ps = pp.tile([128, 512], F32, tag="pp")
                        for kt in range(KT):
                            nc.tensor.matmul(
                                ps[:],
                                xtiles[kt][:, tt * 128:(tt + 1) * 128],
                                wbs[kt // 8][:, (kt % 8) * 512:(kt % 8 + 1) * 512],
                                start=(kt == 0), stop=(kt == KT - 1))
                        nc.scalar.activation(
                            vsb[tt][:, og * 512:(og + 1) * 512], ps[:], AF.Identity)

                # ---- per-head: q-proj+rope, k-proj+rope, attention ----
                def proj_rope(wname, h, tag):
                    wb = qkw.tile([128, KT * 128], BF16, tag="wqk")
                    nc.sync.dma_start(wb[:], aps[wname][h])
                    dst = qkpool.tile([128, BPC * T], BF16, tag=tag)
                    # kt-outer so each 128-col weight slice is loaded into the
                    # PE once and streamed against both batch-pairs.
                    pss = [pp.tile([128, 512], F32, tag="pp", name=f"ps{pr}")
                           for pr in range(PAIRS)]
                    for kt in range(KT):
                        for pr in range(PAIRS):
                            nc.tensor.matmul(
                                pss[pr][:], wb[:, kt * 128:(kt + 1) * 128],
                                xtiles[kt][:, pr * 512:(pr + 1) * 512],
                                start=(kt == 0), stop=(kt == KT - 1))
                    for pr in range(PAIRS):
                        ps = pss[pr]
                        qs = st3.tile([128, 512], BF16, tag="qs")
                        nc.scalar.activation(qs[:], ps[:], AF.Identity)
                        rq = rp.tile([128, 512], F32, tag="rq")
                        nc.tensor.matmul(rq[:], ct["rmatT"][:], qs[:],
                                         start=True, stop=True)
                        rqs = st3.tile([128, 512], BF16, tag="rqs")
                        nc.scalar.activation(rqs[:], rq[:], AF.Identity)
                        t1 = st.tile([128, 512], BF16, tag="t1")
                        nc.vector.tensor_mul(t1[:], qs[:], ct["cos2"][:])
                        t2 = st.tile([128, 512], BF16, tag="t2")
                        nc.vector.tensor_mul(t2[:], rqs[:], ct["sin2"][:])
                        nc.vector.tensor_add(
                            dst[:, pr * 512:(pr + 1) * 512], t1[:], t2[:])
                    return dst

                for h in range(H):
                    qh = proj_rope("wq", h, "q")
                    kh = proj_rope("wk", h, "k")
                    for b in range(BPC):
                        # Causal skip: ktile1 (keys 128..255) is fully masked
                        # for queries 0..127, so score/exp/AV only cover
                        # [k0 x q-all | k1 x q-high] = 384 columns, not 512.
                        # em free layout: [0:256] = k-tile0 vs all 256 q;
                        # [256:384] = k-tile1 vs q in [128,256).
                        sc = scp.tile([128, 384], F32, tag="sc")
                        nc.tensor.matmul(
                            sc[:, 0:256],
                            kh[:, b * 256:b * 256 + 128],
                            qh[:, b * 256:(b + 1) * 256],
                            start=True, stop=True)
                        nc.tensor.matmul(
                            sc[:, 256:384],
                            kh[:, b * 256 + 128:b * 256 + 256],
                            qh[:, b * 256 + 128:b * 256 + 256],
                            start=True, stop=True)
                        ex = st3.tile([128, 384], BF16, tag="ex")
                        nc.scalar.activation(ex[:], sc[:], AF.Exp, scale=SCALE)
                        em = st3.tile([128, 384], BF16, tag="em")
                        tri = ct["mask2"][:, 0:128]       # k<=q triangle
                        nc.vector.tensor_mul(em[:, 0:128], ex[:, 0:128], tri)
                        nc.vector.tensor_copy(em[:, 128:256], ex[:, 128:256])
                        nc.vector.tensor_mul(em[:, 256:384], ex[:, 256:384], tri)
                        se = sep.tile([1, 256], F32, tag="serb")
                        nc.tensor.matmul(se[:], ct["onescol"][:], em[:, 0:256],
                                         start=True, stop=False)
                        nc.tensor.matmul(se[:, 128:256], ct["onescol"][:],
                                         em[:, 256:384],
                                         start=False, stop=True)
                        rr = st.tile([1, 256], F32R, tag="rr")
                        with nc.allow_low_precision(reason="f32r rhs for bcast mm"):
                            nc.vector.reciprocal(rr[:], se[:])
                        # broadcast 1/sum across partitions on the idle GpSimd
                        # engine instead of a K=1 matmul + ScalarE copy
                        rbs = st.tile([128, 256], F32R, tag="rbs")
                        nc.gpsimd.partition_broadcast(rbs[:], rr[:])
                        yps = yp.tile([128, 256], F32, tag="yps")
                        nc.tensor.matmul(yps[:], vsb[b * 2][:, h * 128:(h + 1) * 128],
                                         em[:, 0:256], start=True, stop=False)
                        nc.tensor.matmul(yps[:, 128:256],
                                         vsb[b * 2 + 1][:, h * 128:(h + 1) * 128],
                                         em[:, 256:384], start=False, stop=True)
                        nc.vector.tensor_mul(ysb[h][:, b * 256:(b + 1) * 256],
                                             yps[:], rbs[:].bitcast(F32))

                # ---- output projection ----
                for o in range(16):
                    wb = ow.tile([128, KT * 128], BF16, tag="wob")
                    nc.sync.dma_start(wb[:], aps["wo"][o])
                    pss = [pp.tile([128, 512], F32, tag="pp", name=f"pso{pr}")
                           for pr in range(PAIRS)]
                    for c in range(KT):
                        for pr in range(PAIRS):
                            nc.tensor.matmul(
                                pss[pr][:], wb[:, c * 128:(c + 1) * 128],
                                ysb[c][:, pr * 512:(pr + 1) * 512],
                                start=(c == 0), stop=(c == KT - 1))
                    for pr in range(PAIRS):
                        os_ = st3.tile([128, 512], F32, tag="os")
                        nc.scalar.activation(os_[:], pss[pr][:], AF.Identity)
                        nc.sync.dma_start(
                            out_d[o * 128:(o + 1) * 128,
                                  pr * 512:(pr + 1) * 512], os_[:])

            if loop_n == 1:
                emit_body()
            else:
                hint = (mybir.EngineType.PE, mybir.EngineType.Activation,
                        mybir.EngineType.DVE, mybir.EngineType.SP,
                        mybir.EngineType.Pool)
                with tc.For_i(0, loop_n, 1, hint_engines=hint):
                    emit_body()
    nc.compile()
    return nc


def _host_consts():
    inv_freq = 1.0 / (10000.0 ** (np.arange(0, D, 2, dtype=np.float32) / D))
    t = np.arange(T, dtype=np.float32)
    freqs = np.outer(t, inv_freq)                      # [T, 64]
    emb = np.concatenate([freqs, freqs], axis=-1)      # [T, 128]
    cosT = np.cos(emb).T.astype(np.float32)            # [128, 256]
    sinT = np.sin(emb).T.astype(np.float32)
    cos2 = np.concatenate([cosT, cosT], axis=1)        # [128, 512]
    sin2 = np.concatenate([sinT, sinT], axis=1)
    rmat = np.zeros((128, 128), dtype=np.float32)      # R @ q = rotate_half(q)
    for d in range(64):
        rmat[d, d + 64] = -1.0
        rmat[d + 64, d] = 1.0
    rmatT = rmat.T.copy()
    mask2 = np.zeros((128, 512), dtype=np.float32)
    k_idx = np.arange(128)[:, None]
    q_idx = np.arange(256)[None, :]
    mask2[:, 0:256] = (k_idx <= q_idx).astype(np.float32)
    mask2[:, 256:512] = ((k_idx + 128) <= q_idx).astype(np.float32)
    return {
        "cos2": cos2.astype(NPBF), "sin2": sin2.astype(NPBF),
        "rmatT": rmatT.astype(NPBF), "mask2": mask2.astype(NPBF),
        "onescol": np.ones((128, 1), NPBF),
        "ones1x": np.ones((1, 128), np.float32),
    }


def _tile_w(w, blk):
    """w [2048,2048] fp32 -> [2048//blk, 128, 16*blk] bf16 with
    out[ob, p, kt*blk + n] = w[ob*blk + n, kt*128 + p]."""
    nblk = 2048 // blk
    t = w.reshape(nblk, blk, KT, 128).transpose(0, 3, 2, 1)
    return np.ascontiguousarray(t.reshape(nblk, 128, KT * blk)).astype(NPBF)


def _host_weights(wq, wk, wv, wo):
    return {
        "wq": _tile_w(np.asarray(wq, np.float32), 128),
        "wk": _tile_w(np.asarray(wk, np.float32), 128),
        "wv": _tile_w(np.asarray(wv, np.float32), 512),
        "wo": _tile_w(np.asarray(wo, np.float32), 128),
    }


def _host_x(x):
    """x [B,T,C] fp32 -> per-core [KT, 128, BPC*T] bf16 list."""
    xb = np.asarray(x, np.float32).astype(NPBF)
    outs = []
    for c in range(NCORES):
        xs = xb[c * BPC:(c + 1) * BPC]                      # [4,256,2048]
        xt = xs.transpose(2, 0, 1).reshape(KT, 128, BPC * T)
        outs.append(np.ascontiguousarray(xt))
    return outs


def kernel(x, wq, wk, wv, wo):
    if "nc" not in _CACHE:
        _CACHE["nc"] = _build()
    nc = _CACHE["nc"]
    shared = {**_host_weights(wq, wk, wv, wo), **_host_consts()}
    xts = _host_x(x)
    in_maps = [{"xt": xts[c], **shared} for c in range(NCORES)]
    res = run_bass_kernel_spmd(nc, in_maps, core_ids=list(range(NCORES)))
    outs = []
    for c in range(NCORES):
        o = res.results[c]["out"]                      # [2048, 1024]
        o = o.reshape(C, PAIRS, 2, T)                  # [c, pair, b, t]
        o = o.transpose(1, 2, 3, 0).reshape(BPC, T, C)
        outs.append(o)
    return np.concatenate(outs, axis=0).astype(np.float32)
